# revision 5
# baseline (speedup 1.0000x reference)
"""Trainium2 raw-Bass kernel for nn_MidLoss (segment-mean MSE loss).

Algebraic identity (per segment s with rows x_i and mean mu_s):
    sum_i ||x_i - mu_s||^2 = sum_i ||x_i||^2 - ||colsum_s||^2 / L_s
so  loss = (SSQ - corr) / (N * D) with
    SSQ  = sum x^2
    corr = sum_s ||colsum_s / sqrt(L_s)||^2

Rows are sharded across 8 NeuronCores at segment boundaries; each core
computes partial statistics in ONE pass over its shard (memory-bound);
the host sums partials in float64.

Per-core pipeline (raw Bass, no TileContext — avoids Tile's scheduling
overhead around the stream):
  - SWDGE DMA streams x fp32 HBM -> bf16 SBUF supertiles (cast in-DMA)
  - PE: per 128-col slice, one matmul X^T M into PSUM (M = per-segment
    indicator columns scaled 1/sqrt(L)); accumulation groups are
    supertile-local (segments never straddle supertiles).
  - Act engine: SSQ via Square activation with accum_out — one
    instruction per supertile, no PE Gram. (The 128x-wider Gram matmuls
    drove the package activity monitor into a 50%-duty throttle that
    cut the DMA stream from ~397 to ~340 GB/s.)
  - Vector engine: PSUM cs -> SBUF drains; the first PSUM bank drains
    (and DMAs out) mid-stream, overlapped.
  - First/last supertiles are loaded as 4 quarter-DMAs: earlier first
    packets, and a ~1/4-supertile compute tail after the final byte.

Synchronization notes (hard-won):
  - One DMA-completion semaphore per SBUF slot (tile-style). A DMA's
    "+16" arrives as per-engine increments, so a single cumulative
    semaphore races: fast engines run ahead on later DMAs' portions
    and can reach 16*(n+1) while supertile n is still in flight on a
    slow engine. Quarter DMAs get dedicated semaphores for the same
    reason.
  - Semaphore increments ride then_inc (@complete) on the producing
    instruction; a standalone sem_inc can fire at sequencer-issue time
    before the datapath has committed.
"""

import os
import sys

for _p in ("/opt/trn_rl_repo", "/root/.axon_site/_ro/trn_rl_repo"):
    if os.path.isdir(_p) and _p not in sys.path:
        sys.path.insert(0, _p)

import numpy as np
import ml_dtypes

import concourse.bacc as bacc
from concourse import mybir
from concourse.bass_utils import run_bass_kernel_spmd

N_CORES = 8
D = 128
G_CANDIDATES = (16, 8, 32, 4, 64, 2, 128, 1)
NBUF = 12  # SBUF supertile slots (bf16 [128, G*D] each)


def _structure(lengths, n_cores=N_CORES):
    """Host-side plan: shard segments, pick layout, build membership info."""
    lengths = np.asarray(lengths, dtype=np.int64)
    S = int(lengths.shape[0])
    offs = np.zeros(S + 1, dtype=np.int64)
    np.cumsum(lengths, out=offs[1:])
    N = int(offs[-1])

    splits = [0]
    for c in range(1, n_cores):
        target = c * N / n_cores
        s = int(np.argmin(np.abs(offs - target)))
        splits.append(s)
    splits.append(S)
    for c in range(n_cores):
        if splits[c + 1] <= splits[c]:
            return None, True
    shard_rows = [int(offs[splits[c + 1]] - offs[splits[c]]) for c in range(n_cores)]
    if len(set(shard_rows)) != 1:
        return None, True
    R = shard_rows[0]

    g_pref = int(os.environ.get("MIDLOSS_G", "0"))
    G = None
    for g in ((g_pref,) if g_pref else ()) + G_CANDIDATES:
        if R % (128 * g) == 0 and np.all(lengths % g == 0):
            G = g
            break
    if G is None:
        return None, True
    rows_super = 128 * G
    n_super = R // rows_super

    cores = []
    for c in range(n_cores):
        s_lo, s_hi = splits[c], splits[c + 1]
        seg_off = offs[s_lo:s_hi + 1] - offs[s_lo]
        seg_len = lengths[s_lo:s_hi]
        s_count = s_hi - s_lo
        inv_sqrt_l = (1.0 / np.sqrt(seg_len.astype(np.float64))).astype(np.float32)

        supers = []
        memb_cols = []
        col_off = 0
        ok = True
        for n in range(n_super):
            lo, hi = n * rows_super, (n + 1) * rows_super
            s0 = int(np.searchsorted(seg_off, lo, side="right") - 1)
            s1 = int(np.searchsorted(seg_off, hi, side="left") - 1)
            k = s1 - s0 + 1
            pstart = lo + G * np.arange(128, dtype=np.int64)
            pseg = np.searchsorted(seg_off, pstart, side="right") - 1
            for j in range(k):
                col = np.where(pseg == s0 + j, inv_sqrt_l[s0 + j], 0.0)
                memb_cols.append(col.astype(np.float32))
            supers.append((s0, k, col_off))
            col_off += k
            # raw kernel uses per-supertile cs accumulation groups
            # (start at g==0, stop at g==G-1); a segment straddling a
            # supertile boundary would need cross-group accumulation.
            if n > 0 and supers[n - 1][0] + supers[n - 1][1] > s0:
                ok = False
        if not ok:
            return None, True
        memb = np.stack(memb_cols, axis=1)
        cores.append(dict(s_lo=s_lo, s_hi=s_hi, s_count=s_count,
                          supers=supers, memb=memb,
                          row_lo=int(offs[s_lo]), row_hi=int(offs[s_hi])))

    sig0 = (cores[0]["s_count"], tuple(cores[0]["supers"]))
    for c in range(1, n_cores):
        if (cores[c]["s_count"], tuple(cores[c]["supers"])) != sig0:
            return None, True
    s_count = cores[0]["s_count"]
    if s_count > 512:
        return None, True
    n_memb_cols = cores[0]["memb"].shape[1]
    if n_memb_cols != s_count:
        # per-supertile groups need cs columns == segments (no straddle)
        return None, True

    plan = dict(R=R, G=G, n_super=n_super, s_count=s_count,
                n_memb_cols=n_memb_cols,
                supers=cores[0]["supers"], cores=cores, N=N)
    return plan, False


def _build_nc(R, G, n_super, s_count, n_memb_cols, supers):
    f32 = mybir.dt.float32
    bf16 = mybir.dt.bfloat16

    FB = G * D
    # quarter-split the first and last supertiles: first so packets start
    # flowing after ~1/4 of the descriptor generation, last so the PE/DVE
    # tail after the final byte is ~1/4 of a supertile.
    NQ = 4 if G % 4 == 0 and n_super > 2 else 1
    QB = FB // NQ
    split = (0, n_super - 1) if NQ > 1 else ()
    # ssq accumulator columns: one per un-split supertile, one per quarter
    n_acc = (n_super - len(split)) + NQ * len(split)
    ycols = s_count + n_acc

    nc = bacc.Bacc()
    x = nc.dram_tensor("x", [R, D], f32, kind="ExternalInput")
    memb = nc.dram_tensor("memb", [128, n_memb_cols], bf16, kind="ExternalInput")
    y = nc.dram_tensor("y", [128, ycols], f32, kind="ExternalOutput")

    xv = x[:].rearrange("(n p g) d -> n p (g d)", p=128, g=G)
    if NQ > 1:
        xq = x[:].rearrange("(n p q r) d -> n q p (r d)", p=128, q=NQ, r=G // NQ)

    # split cs columns across two PSUM banks at the column where the
    # second half of the stream begins; bank A is drained mid-stream.
    half = n_super // 2
    ch = supers[half][2] if n_super > 1 else 0
    # every supertile's column range must sit entirely in one bank
    if ch > 0:
        for (s0, k, c0) in supers:
            if c0 < ch < c0 + k:
                ch = 0
                break
    nA = max(ch, 1)  # avoid zero-size tensors; unused if ch == 0

    xb = nc.alloc_sbuf_tensor("xb", [128, NBUF * FB], bf16)
    memb_sb = nc.alloc_sbuf_tensor("memb_sb", [128, n_memb_cols], bf16)
    out_sb = nc.alloc_sbuf_tensor("out_sb", [128, ycols], f32)
    # write-only sink for the Square activation's elementwise output
    # (only accum_out is consumed); overlapping writes are harmless.
    trash = nc.alloc_sbuf_tensor("trash", [128, FB], bf16)
    psum_csA = nc.alloc_psum_tensor("psum_csA", [128, nA], f32)
    psum_csB = nc.alloc_psum_tensor("psum_csB", [128, s_count - ch], f32)

    hw_sem = nc.alloc_semaphore("hw_sem")
    pe_sem = nc.alloc_semaphore("pe_sem")
    sc_sem = nc.alloc_semaphore("sc_sem")
    fin_sem = nc.alloc_semaphore("fin_sem")
    cs1_sem = nc.alloc_semaphore("cs1_sem")
    out_sem = nc.alloc_semaphore("out_sem")
    # Per-quarter semaphores for the split supertiles. A quarter DMA tags
    # few DMA engines, so a later quarter can fully complete before an
    # earlier one; cumulative counts on one semaphore would race.
    q_sems = [nc.alloc_semaphore(f"q_sem{i}") for i in range(NQ)]
    fq_sems = [nc.alloc_semaphore(f"fq_sem{i}") for i in range(NQ)]
    # One DMA-completion semaphore per SBUF slot, tile-style. A DMA's
    # "+16" arrives as per-engine increments, so a single cumulative
    # semaphore races: fast engines run ahead on later DMAs' portions
    # and can reach 16*(n+1) while supertile n is still in flight on
    # a slow engine. Per-slot counts are exact.
    slot_sems = [nc.alloc_semaphore(f"slot_sem{i}") for i in range(NBUF)]

    def cs_dst(c0, k):
        if c0 < ch:
            return psum_csA[:, c0:c0 + k]
        return psum_csB[:, c0 - ch:c0 - ch + k]

    qsems_of = {0: fq_sems, n_super - 1: q_sems}

    # ---- sync engine: memb load, then the two output DMAs ----
    nc.sync.dma_start(out=memb_sb[:], in_=memb[:]).then_inc(hw_sem, 16)
    if ch > 0:
        nc.sync.wait_ge(cs1_sem, 1)
        nc.sync.dma_start(out=y[:, 0:ch], in_=out_sb[:, 0:ch]).then_inc(
            out_sem, 16)
    nc.sync.wait_ge(fin_sem, 2)
    nc.sync.dma_start(
        out=y[:, ch:ycols], in_=out_sb[:, ch:ycols]
    ).then_inc(out_sem, 16)
    # No trailing wait on out_sem: the NEFF epilogue drains quiesce the
    # HWDGE ring before execution-complete (same contract the Tile
    # framework relies on), and waiting here holds the end-of-kernel
    # barrier hostage for ~1 us.

    # ---- gpsimd: the x stream (slot freed when PE and Act both done) ----
    for n in range(n_super):
        if n >= NBUF:
            nc.gpsimd.wait_ge(pe_sem, n - NBUF + 1)
            nc.gpsimd.wait_ge(sc_sem, n - NBUF + 1)
        slot = (n % NBUF) * FB
        if n in split:
            for q in range(NQ):
                nc.gpsimd.dma_start(
                    out=xb[:, slot + q * QB: slot + (q + 1) * QB],
                    in_=xq[n][q],
                ).then_inc(qsems_of[n][q], 16)
        else:
            nc.gpsimd.dma_start(
                out=xb[:, slot: slot + FB], in_=xv[n]
            ).then_inc(slot_sems[n % NBUF], 16)

    # ---- tensor: membership matmuls only (cs columns) ----
    nc.tensor.wait_ge(hw_sem, 16)
    pe_rounds = [0] * NBUF
    mmc = None
    for n in range(n_super):
        s0, k, c0 = supers[n]
        slot = (n % NBUF) * FB
        nq = NQ if n in split else 1
        for q in range(nq):
            if n in split:
                nc.tensor.wait_ge(qsems_of[n][q], 16)
            else:
                si = n % NBUF
                pe_rounds[si] += 1
                nc.tensor.wait_ge(slot_sems[si], 16 * pe_rounds[si])
            for g in range(q * (G // nq), (q + 1) * (G // nq)):
                st = xb[:, slot + g * D: slot + (g + 1) * D]
                mmc = nc.tensor.matmul(
                    cs_dst(c0, k), lhsT=st,
                    rhs=memb_sb[:, c0:c0 + k],
                    start=(g == 0), stop=(g == G - 1),
                )
        mmc.then_inc(pe_sem, 1)

    # ---- scalar (Act): ssq via Square activation with accum_out ----
    # All semaphore increments ride then_inc (@complete) on the act
    # itself: a standalone sem_inc can fire at sequencer-issue time,
    # before the datapath finishes reading xb / writing accum_out.
    sc_rounds = [0] * NBUF
    acc_col = 0
    SQ = mybir.ActivationFunctionType.Square
    for n in range(n_super):
        slot = (n % NBUF) * FB
        act = None
        if n in split:
            for q in range(NQ):
                nc.scalar.wait_ge(qsems_of[n][q], 16)
                seg = xb[:, slot + q * QB: slot + (q + 1) * QB]
                act = nc.scalar.activation(
                    out=trash[:, 0:QB], in_=seg, func=SQ,
                    accum_out=out_sb[:, s_count + acc_col:
                                     s_count + acc_col + 1])
                acc_col += 1
        else:
            si = n % NBUF
            sc_rounds[si] += 1
            nc.scalar.wait_ge(slot_sems[si], 16 * sc_rounds[si])
            seg = xb[:, slot: slot + FB]
            act = nc.scalar.activation(
                out=trash[:], in_=seg, func=SQ,
                accum_out=out_sb[:, s_count + acc_col:
                                 s_count + acc_col + 1])
            acc_col += 1
        act.then_inc(sc_sem, 1)
    assert acc_col == n_acc
    # sc_sem increments fire at act completion, so this wait passes only
    # once the last accum column is written; the inc after it is safe.
    nc.scalar.wait_ge(sc_sem, n_super)
    nc.scalar.sem_inc(fin_sem, 1)

    # ---- vector: cs PSUM -> SBUF drains ----
    if ch > 0:
        nc.vector.wait_ge(pe_sem, half)
        nc.vector.tensor_copy(
            out=out_sb[:, 0:ch], in_=psum_csA[:, 0:ch]
        ).then_inc(cs1_sem, 1)
    nc.vector.wait_ge(pe_sem, n_super)
    nc.vector.tensor_copy(
        out=out_sb[:, ch:s_count], in_=psum_csB[:]
    ).then_inc(fin_sem, 1)

    nc.compile()
    return nc


_CACHE = {}


def _get_nc(plan):
    key = (plan["R"], plan["G"], plan["n_super"], plan["s_count"],
           plan["n_memb_cols"], tuple(plan["supers"]))
    nc = _CACHE.get(key)
    if nc is None:
        nc = _build_nc(plan["R"], plan["G"], plan["n_super"], plan["s_count"],
                       plan["n_memb_cols"], plan["supers"])
        _CACHE[key] = nc
    return nc


def _run_spmd(plan, x_np, trace=False):
    nc = _get_nc(plan)
    in_maps = []
    for c in range(N_CORES):
        info = plan["cores"][c]
        shard = np.ascontiguousarray(x_np[info["row_lo"]:info["row_hi"]])
        in_maps.append({
            "x": shard,
            "memb": info["memb"].astype(ml_dtypes.bfloat16),
        })
    last_err = None
    for attempt in range(3):
        try:
            res = run_bass_kernel_spmd(nc, in_maps,
                                       core_ids=list(range(N_CORES)),
                                       trace=trace)
            break
        except Exception as e:
            last_err = e
    else:
        raise last_err
    s_count = plan["s_count"]
    partials = []
    for c in range(N_CORES):
        yv = np.asarray(res.results[c]["y"], dtype=np.float64)
        ssq = float(yv[:, s_count:].sum())
        corr = float((yv[:, :s_count] ** 2).sum())
        partials.append(ssq - corr)
    return partials, res


def _numpy_fallback(x_np, lengths):
    lengths = np.asarray(lengths, dtype=np.int64)
    offs = np.concatenate([[0], np.cumsum(lengths)])
    x = x_np.astype(np.float64)
    ssq = float((x * x).sum())
    corr = 0.0
    for s in range(len(lengths)):
        cs = x[offs[s]:offs[s + 1]].sum(axis=0)
        corr += float((cs * cs).sum()) / float(lengths[s])
    return np.float32((ssq - corr) / x.size)


def kernel(inputs, lengths):
    x_np = np.asarray(inputs, dtype=np.float32)
    lengths_np = np.asarray(lengths)
    plan, fallback = _structure(lengths_np)
    if fallback:
        return _numpy_fallback(x_np, lengths_np)
    partials, _ = _run_spmd(plan, x_np)
    total = float(np.sum(np.asarray(partials, dtype=np.float64)))
    loss = total / (plan["N"] * D)
    return np.asarray(loss, dtype=np.float32)


# revision 7
# speedup vs baseline: 1.0044x; 1.0044x over previous
"""Trainium2 raw-Bass kernel for nn_MidLoss (segment-mean MSE loss).

Algebraic identity (per segment s with rows x_i and mean mu_s):
    sum_i ||x_i - mu_s||^2 = sum_i ||x_i||^2 - ||colsum_s||^2 / L_s
so  loss = (SSQ - corr) / (N * D) with
    SSQ  = sum x^2
    corr = sum_s ||colsum_s / sqrt(L_s)||^2

Rows are sharded across 8 NeuronCores at segment boundaries; each core
computes partial statistics in ONE pass over its shard (memory-bound);
the host sums partials in float64.

Per-core pipeline (raw Bass, no TileContext — avoids Tile's scheduling
overhead around the stream):
  - SWDGE DMA streams x fp32 HBM -> bf16 SBUF supertiles (cast in-DMA)
  - PE: per 128-col slice, one matmul X^T M into PSUM (M = per-segment
    indicator columns scaled 1/sqrt(L)); accumulation groups are
    supertile-local (segments never straddle supertiles).
  - Act engine: SSQ via Square activation with accum_out — one
    instruction per supertile, no PE Gram. (The 128x-wider Gram matmuls
    drove the package activity monitor into a 50%-duty throttle that
    cut the DMA stream from ~397 to ~340 GB/s.)
  - Vector engine: PSUM cs -> SBUF drains; the first PSUM bank drains
    (and DMAs out) mid-stream, overlapped.
  - First/last supertiles are loaded as 4 quarter-DMAs: earlier first
    packets, and a ~1/4-supertile compute tail after the final byte.

Synchronization notes (hard-won):
  - One DMA-completion semaphore per SBUF slot (tile-style). A DMA's
    "+16" arrives as per-engine increments, so a single cumulative
    semaphore races: fast engines run ahead on later DMAs' portions
    and can reach 16*(n+1) while supertile n is still in flight on a
    slow engine. Quarter DMAs get dedicated semaphores for the same
    reason.
  - Semaphore increments ride then_inc (@complete) on the producing
    instruction; a standalone sem_inc can fire at sequencer-issue time
    before the datapath has committed.
"""

import os
import sys

for _p in ("/opt/trn_rl_repo", "/root/.axon_site/_ro/trn_rl_repo"):
    if os.path.isdir(_p) and _p not in sys.path:
        sys.path.insert(0, _p)

import numpy as np
import ml_dtypes

import concourse.bacc as bacc
from concourse import mybir
from concourse.bass_utils import run_bass_kernel_spmd

N_CORES = 8
D = 128
G_CANDIDATES = (16, 8, 32, 4, 64, 2, 128, 1)
NBUF = 12  # SBUF supertile slots (bf16 [128, G*D] each)


def _structure(lengths, n_cores=N_CORES):
    """Host-side plan: shard segments, pick layout, build membership info."""
    lengths = np.asarray(lengths, dtype=np.int64)
    S = int(lengths.shape[0])
    offs = np.zeros(S + 1, dtype=np.int64)
    np.cumsum(lengths, out=offs[1:])
    N = int(offs[-1])

    splits = [0]
    for c in range(1, n_cores):
        target = c * N / n_cores
        s = int(np.argmin(np.abs(offs - target)))
        splits.append(s)
    splits.append(S)
    for c in range(n_cores):
        if splits[c + 1] <= splits[c]:
            return None, True
    shard_rows = [int(offs[splits[c + 1]] - offs[splits[c]]) for c in range(n_cores)]
    if len(set(shard_rows)) != 1:
        return None, True
    R = shard_rows[0]

    g_pref = int(os.environ.get("MIDLOSS_G", "0"))
    G = None
    for g in ((g_pref,) if g_pref else ()) + G_CANDIDATES:
        if R % (128 * g) == 0 and np.all(lengths % g == 0):
            G = g
            break
    if G is None:
        return None, True
    rows_super = 128 * G
    n_super = R // rows_super

    cores = []
    for c in range(n_cores):
        s_lo, s_hi = splits[c], splits[c + 1]
        seg_off = offs[s_lo:s_hi + 1] - offs[s_lo]
        seg_len = lengths[s_lo:s_hi]
        s_count = s_hi - s_lo
        inv_sqrt_l = (1.0 / np.sqrt(seg_len.astype(np.float64))).astype(np.float32)

        supers = []
        memb_cols = []
        col_off = 0
        ok = True
        for n in range(n_super):
            lo, hi = n * rows_super, (n + 1) * rows_super
            s0 = int(np.searchsorted(seg_off, lo, side="right") - 1)
            s1 = int(np.searchsorted(seg_off, hi, side="left") - 1)
            k = s1 - s0 + 1
            pstart = lo + G * np.arange(128, dtype=np.int64)
            pseg = np.searchsorted(seg_off, pstart, side="right") - 1
            for j in range(k):
                col = np.where(pseg == s0 + j, inv_sqrt_l[s0 + j], 0.0)
                memb_cols.append(col.astype(np.float32))
            supers.append((s0, k, col_off))
            col_off += k
            # raw kernel uses per-supertile cs accumulation groups
            # (start at g==0, stop at g==G-1); a segment straddling a
            # supertile boundary would need cross-group accumulation.
            if n > 0 and supers[n - 1][0] + supers[n - 1][1] > s0:
                ok = False
        if not ok:
            return None, True
        memb = np.stack(memb_cols, axis=1)
        cores.append(dict(s_lo=s_lo, s_hi=s_hi, s_count=s_count,
                          supers=supers, memb=memb,
                          row_lo=int(offs[s_lo]), row_hi=int(offs[s_hi])))

    sig0 = (cores[0]["s_count"], tuple(cores[0]["supers"]))
    for c in range(1, n_cores):
        if (cores[c]["s_count"], tuple(cores[c]["supers"])) != sig0:
            return None, True
    s_count = cores[0]["s_count"]
    if s_count > 512:
        return None, True
    n_memb_cols = cores[0]["memb"].shape[1]
    if n_memb_cols != s_count:
        # per-supertile groups need cs columns == segments (no straddle)
        return None, True

    plan = dict(R=R, G=G, n_super=n_super, s_count=s_count,
                n_memb_cols=n_memb_cols,
                supers=cores[0]["supers"], cores=cores, N=N)
    return plan, False


def _build_nc(R, G, n_super, s_count, n_memb_cols, supers):
    f32 = mybir.dt.float32
    bf16 = mybir.dt.bfloat16

    FB = G * D
    # quarter-split the first and last supertiles: first so packets start
    # flowing after ~1/4 of the descriptor generation, last so the PE/DVE
    # tail after the final byte is ~1/4 of a supertile.
    NQ = 4 if G % 4 == 0 and n_super > 2 else 1
    QB = FB // NQ
    split = (0, n_super - 1) if NQ > 1 else ()
    # ssq accumulator columns: one per un-split supertile, one per quarter
    n_acc = (n_super - len(split)) + NQ * len(split)
    ycols = s_count + n_acc

    nc = bacc.Bacc()
    x = nc.dram_tensor("x", [R, D], f32, kind="ExternalInput")
    memb = nc.dram_tensor("memb", [128, n_memb_cols], bf16, kind="ExternalInput")
    y = nc.dram_tensor("y", [128, ycols], f32, kind="ExternalOutput")

    xv = x[:].rearrange("(n p g) d -> n p (g d)", p=128, g=G)
    if NQ > 1:
        xq = x[:].rearrange("(n p q r) d -> n q p (r d)", p=128, q=NQ, r=G // NQ)

    # split cs columns across two PSUM banks at the column where the
    # second half of the stream begins; bank A is drained mid-stream.
    half = n_super // 2
    ch = supers[half][2] if n_super > 1 else 0
    # every supertile's column range must sit entirely in one bank
    if ch > 0:
        for (s0, k, c0) in supers:
            if c0 < ch < c0 + k:
                ch = 0
                break
    nA = max(ch, 1)  # avoid zero-size tensors; unused if ch == 0

    xb = nc.alloc_sbuf_tensor("xb", [128, NBUF * FB], bf16)
    memb_sb = nc.alloc_sbuf_tensor("memb_sb", [128, n_memb_cols], bf16)
    out_sb = nc.alloc_sbuf_tensor("out_sb", [128, ycols], f32)
    # write-only sink for the Square activation's elementwise output
    # (only accum_out is consumed); overlapping writes are harmless.
    trash = nc.alloc_sbuf_tensor("trash", [128, FB], bf16)
    psum_csA = nc.alloc_psum_tensor("psum_csA", [128, nA], f32)
    psum_csB = nc.alloc_psum_tensor("psum_csB", [128, s_count - ch], f32)

    hw_sem = nc.alloc_semaphore("hw_sem")
    pe_sem = nc.alloc_semaphore("pe_sem")
    sc_sem = nc.alloc_semaphore("sc_sem")
    fin_sem = nc.alloc_semaphore("fin_sem")
    cs1_sem = nc.alloc_semaphore("cs1_sem")
    out_sem = nc.alloc_semaphore("out_sem")
    # Per-quarter semaphores for the split supertiles. A quarter DMA tags
    # few DMA engines, so a later quarter can fully complete before an
    # earlier one; cumulative counts on one semaphore would race.
    q_sems = [nc.alloc_semaphore(f"q_sem{i}") for i in range(NQ)]
    fq_sems = [nc.alloc_semaphore(f"fq_sem{i}") for i in range(NQ)]
    # One DMA-completion semaphore per SBUF slot, tile-style. A DMA's
    # "+16" arrives as per-engine increments, so a single cumulative
    # semaphore races: fast engines run ahead on later DMAs' portions
    # and can reach 16*(n+1) while supertile n is still in flight on
    # a slow engine. Per-slot counts are exact.
    slot_sems = [nc.alloc_semaphore(f"slot_sem{i}") for i in range(NBUF)]

    def cs_dst(c0, k):
        if c0 < ch:
            return psum_csA[:, c0:c0 + k]
        return psum_csB[:, c0 - ch:c0 - ch + k]

    qsems_of = {0: fq_sems, n_super - 1: q_sems}

    # ---- sync engine: memb load, then the two output DMAs ----
    nc.sync.dma_start(out=memb_sb[:], in_=memb[:]).then_inc(hw_sem, 16)
    if ch > 0:
        nc.sync.wait_ge(cs1_sem, 1)
        nc.sync.dma_start(out=y[:, 0:ch], in_=out_sb[:, 0:ch]).then_inc(
            out_sem, 16)
    nc.sync.wait_ge(fin_sem, 2)
    nc.sync.dma_start(
        out=y[:, ch:ycols], in_=out_sb[:, ch:ycols]
    ).then_inc(out_sem, 16)
    # No trailing wait on out_sem: the NEFF epilogue drains quiesce the
    # HWDGE ring before execution-complete (same contract the Tile
    # framework relies on), and waiting here holds the end-of-kernel
    # barrier hostage for ~1 us.

    # ---- gpsimd: the x stream (slot freed when PE and Act both done) ----
    for n in range(n_super):
        if n >= NBUF:
            nc.gpsimd.wait_ge(pe_sem, n - NBUF + 1)
            nc.gpsimd.wait_ge(sc_sem, n - NBUF + 1)
        slot = (n % NBUF) * FB
        if n in split:
            for q in range(NQ):
                nc.gpsimd.dma_start(
                    out=xb[:, slot + q * QB: slot + (q + 1) * QB],
                    in_=xq[n][q],
                ).then_inc(qsems_of[n][q], 16)
        else:
            nc.gpsimd.dma_start(
                out=xb[:, slot: slot + FB], in_=xv[n]
            ).then_inc(slot_sems[n % NBUF], 16)

    # ---- tensor: membership matmuls only (cs columns) ----
    nc.tensor.wait_ge(hw_sem, 16)
    pe_rounds = [0] * NBUF
    mmc = None
    for n in range(n_super):
        s0, k, c0 = supers[n]
        slot = (n % NBUF) * FB
        nq = NQ if n in split else 1
        for q in range(nq):
            if n in split:
                nc.tensor.wait_ge(qsems_of[n][q], 16)
            else:
                si = n % NBUF
                pe_rounds[si] += 1
                nc.tensor.wait_ge(slot_sems[si], 16 * pe_rounds[si])
            for g in range(q * (G // nq), (q + 1) * (G // nq)):
                st = xb[:, slot + g * D: slot + (g + 1) * D]
                mmc = nc.tensor.matmul(
                    cs_dst(c0, k), lhsT=st,
                    rhs=memb_sb[:, c0:c0 + k],
                    start=(g == 0), stop=(g == G - 1),
                )
        mmc.then_inc(pe_sem, 1)

    # ---- scalar (Act): ssq via Square activation with accum_out ----
    # All semaphore increments ride then_inc (@complete) on the act
    # itself: a standalone sem_inc can fire at sequencer-issue time,
    # before the datapath finishes reading xb / writing accum_out.
    sc_rounds = [0] * NBUF
    acc_col = 0
    SQ = mybir.ActivationFunctionType.Square
    for n in range(n_super):
        slot = (n % NBUF) * FB
        act = None
        if n in split:
            for q in range(NQ):
                nc.scalar.wait_ge(qsems_of[n][q], 16)
                seg = xb[:, slot + q * QB: slot + (q + 1) * QB]
                act = nc.scalar.activation(
                    out=trash[:, 0:QB], in_=seg, func=SQ,
                    accum_out=out_sb[:, s_count + acc_col:
                                     s_count + acc_col + 1])
                acc_col += 1
        else:
            si = n % NBUF
            sc_rounds[si] += 1
            nc.scalar.wait_ge(slot_sems[si], 16 * sc_rounds[si])
            seg = xb[:, slot: slot + FB]
            act = nc.scalar.activation(
                out=trash[:], in_=seg, func=SQ,
                accum_out=out_sb[:, s_count + acc_col:
                                 s_count + acc_col + 1])
            acc_col += 1
        act.then_inc(sc_sem, 1)
    assert acc_col == n_acc
    # sc_sem increments fire at act completion, so this wait passes only
    # once the last accum column is written; the inc after it is safe.
    nc.scalar.wait_ge(sc_sem, n_super)
    nc.scalar.sem_inc(fin_sem, 1)

    # ---- vector: cs PSUM -> SBUF drains ----
    if ch > 0:
        nc.vector.wait_ge(pe_sem, half)
        nc.vector.tensor_copy(
            out=out_sb[:, 0:ch], in_=psum_csA[:, 0:ch]
        ).then_inc(cs1_sem, 1)
    nc.vector.wait_ge(pe_sem, n_super)
    nc.vector.tensor_copy(
        out=out_sb[:, ch:s_count], in_=psum_csB[:]
    ).then_inc(fin_sem, 1)

    nc.compile()
    return nc


_CACHE = {}


def _get_nc(plan):
    key = (plan["R"], plan["G"], plan["n_super"], plan["s_count"],
           plan["n_memb_cols"], tuple(plan["supers"]))
    nc = _CACHE.get(key)
    if nc is None:
        nc = _build_nc(plan["R"], plan["G"], plan["n_super"], plan["s_count"],
                       plan["n_memb_cols"], plan["supers"])
        _CACHE[key] = nc
    return nc


def _run_spmd(plan, x_np, trace=False):
    nc = _get_nc(plan)
    in_maps = []
    for c in range(N_CORES):
        info = plan["cores"][c]
        shard = np.ascontiguousarray(x_np[info["row_lo"]:info["row_hi"]])
        in_maps.append({
            "x": shard,
            "memb": info["memb"].astype(ml_dtypes.bfloat16),
        })
    last_err = None
    for attempt in range(3):
        try:
            res = run_bass_kernel_spmd(nc, in_maps,
                                       core_ids=list(range(N_CORES)),
                                       trace=trace)
            break
        except Exception as e:
            last_err = e
    else:
        raise last_err
    s_count = plan["s_count"]
    partials = []
    for c in range(N_CORES):
        yv = np.asarray(res.results[c]["y"], dtype=np.float64)
        ssq = float(yv[:, s_count:].sum())
        corr = float((yv[:, :s_count] ** 2).sum())
        partials.append(ssq - corr)
    return partials, res


def _numpy_fallback(x_np, lengths):
    lengths = np.asarray(lengths, dtype=np.int64)
    offs = np.concatenate([[0], np.cumsum(lengths)])
    x = x_np.astype(np.float64)
    ssq = float((x * x).sum())
    corr = 0.0
    for s in range(len(lengths)):
        cs = x[offs[s]:offs[s + 1]].sum(axis=0)
        corr += float((cs * cs).sum()) / float(lengths[s])
    return np.float32((ssq - corr) / x.size)


def kernel(inputs, lengths):
    x_np = np.asarray(inputs, dtype=np.float32)
    lengths_np = np.asarray(lengths)
    plan, fallback = _structure(lengths_np)
    if fallback:
        return _numpy_fallback(x_np, lengths_np)
    partials, _ = _run_spmd(plan, x_np)
    total = float(np.sum(np.asarray(partials, dtype=np.float64)))
    loss = total / (plan["N"] * D)
    return np.asarray(loss, dtype=np.float32)
